# revision 2
# baseline (speedup 1.0000x reference)
"""Grouped MoE MLP (SwiGLU) for Trainium2, expert-parallel across 8 NeuronCores.

Problem: out = gmm(silu(gmm(x,Wg)) * gmm(x,Wu), Wd) with E=8 experts,
T=8192 tokens pre-sorted by expert, H=2048, I=4096.

Strategy: expert parallelism — core e computes expert e's tokens end-to-end.
The host splits the (ragged) token dim by expert, pads each group to a fixed
capacity C, casts to bf16, and PRE-TILES every tensor into the exact SBUF
layout the device consumes, so every DMA is a plain contiguous 2D copy at
full line rate (2-8 KB lines, minimal descriptors, minimal issue cost).

Device program per core (all shapes hardcoded at build time):
  GEMM1 computes the SwiGLU intermediate TRANSPOSED (interT[I, C]) so that
  GEMM2's contraction dim (I) is already the partition dim — no on-device
  transposes anywhere. bf16 inputs, fp32 PSUM accumulation, bf16 output.

Ramp engineering (the steady-state MM stream is already at the 216 ns/MM
N=512 floor, so the wins are at the edges):
  * ~14 warmup matmuls on a zeroed scratch tile run during the DMA ramp so
    the PE_HAM clock gate reaches K=8/8 (2.4 GHz) before real work arrives.
  * The first weight chunk is one k-slice (64 KB) and the first xt slice is
    one (k, t)-quarter (128 KB), so the first real matmul fires ~2 us
    earlier than with coarse transfers.
  * All t=0 halves of xt stream before t=1 halves, alternating rings by k
    parity, and pair-0's psum groups run t-outer — the PE consumes data in
    exactly the order it lands.
"""

import numpy as np
import ml_dtypes

P = 128          # partition dim
NB = 512         # matmul moving free dim / PSUM bank width (fp32)
E, T, H, I = 8, 8192, 2048, 4096
C_DEFAULT = T // E  # per-expert token capacity
WARMUP = 14      # HAM warmup matmuls

_NC_CACHE = {}


def _build(C, Hd, Id, nb=NB):
    """Build + bacc-compile the per-core Tile program. Returns the Bass module."""
    import concourse.bass as bass  # noqa: F401
    import concourse.tile as tile
    from concourse import bacc, mybir

    bf16 = mybir.dt.bfloat16
    f32 = mybir.dt.float32
    KT = Hd // P       # GEMM1 contraction tiles (over H)
    IT = Id // P       # i-tiles (GEMM1 output partitions / GEMM2 contraction)
    TT = C // nb       # token blocks for GEMM1 moving operand
    T8 = C // P        # token tiles for GEMM2 output partitions
    HB = Hd // nb      # h-blocks for GEMM2 moving operand
    IP = IT // 2       # i-block pairs
    W2 = 2 * P         # pair width in I columns

    nc = bacc.Bacc(
        "TRN2",
        target_bir_lowering=False,
        debug=False,
        enable_asserts=False,
        num_devices=8,
    )
    # Host-pre-tiled layouts (see _prepare):
    #   xT[p, k*C + c]   = x[c, k*P + p]
    #   wg[pp*P+p, k*W2+c] = gate[k*P+p, pp*W2+c]   (same for wu)
    #   wd[h*P+p, k*nb+c]  = down[k*P+p, h*nb+c]
    xT = nc.dram_tensor("xT", [P, KT * C], bf16, kind="ExternalInput").ap()
    wg = nc.dram_tensor("wg", [IP * P, KT * W2], bf16, kind="ExternalInput").ap()
    wu = nc.dram_tensor("wu", [IP * P, KT * W2], bf16, kind="ExternalInput").ap()
    wd = nc.dram_tensor("wd", [HB * P, IT * nb], bf16, kind="ExternalInput").ap()
    out = nc.dram_tensor("out", [C, Hd], bf16, kind="ExternalOutput").ap()

    def k3(ap):
        return ap.rearrange("p (k c) -> p k c", c=C)

    with tile.TileContext(nc) as tc:
        with tc.tile_pool(name="res", bufs=1) as res:
            # SwiGLU intermediate, transposed: interT[p, i*C + c] = inter[c, i*P+p]
            interT = res.tile([P, IT * C], bf16)
            # h=0 block of Wd, prefetched during phase 1 so phase 2 starts hot
            wd0 = res.tile([P, IT * nb], bf16)

            # ps1 spans BOTH phases (6 banks) so the allocator must give ps2
            # disjoint fresh banks — otherwise the first phase-2 matmul
            # inherits a conservative wait on ALL phase-1 matmuls completing.
            with tc.tile_pool(name="ps1", bufs=2, space="PSUM") as ps1:
              # ------------- Phase 1: gate/up GEMMs + SwiGLU -------------
              with tc.tile_pool(name="wrm", bufs=1, space="PSUM") as wrm, \
                 tc.tile_pool(name="p1x", bufs=1) as p1x, \
                 tc.tile_pool(name="w1", bufs=3) as w1, \
                 tc.tile_pool(name="tmp1", bufs=4) as tmp1:
                  # HAM warmup: keep the PE busy during the DMA ramp so the
                  # clock gate opens to 2.4 GHz before the first real matmul.
                  scr = p1x.tile([P, P], bf16)
                  nc.vector.memset(scr[:], 0.0)
                  psw = wrm.tile([P, P], f32)
                  for _ in range(WARMUP):
                      nc.tensor.matmul(psw[:], scr[:], scr[:],
                                       start=True, stop=True)

                  wgp0 = w1.tile([P, KT * W2], bf16, tag="wg")
                  wup0 = w1.tile([P, KT * W2], bf16, tag="wu")
                  xt = p1x.tile([P, KT * C], bf16)
                  # Ramp: the first group consumes (wg k-slice, xt k-t0
                  # slice) pairs in k order; feed both rings so every slice
                  # lands just before the PE needs it.  Weight chunks grow
                  # geometrically (k0 | k1 | k2-3 | k4-7 | k8-15).
                  CH = [(0, 1), (1, 2), (2, 4), (4, 8), (8, KT)]
                  ci = 0
                  for k in range(KT):
                      if ci < len(CH) and CH[ci][0] == k:
                          k0, k1 = CH[ci]
                          nc.sync.dma_start(
                              wgp0[:, k0 * W2:k1 * W2],
                              wg[0:P, k0 * W2:k1 * W2])
                          nc.scalar.dma_start(
                              wup0[:, k0 * W2:k1 * W2],
                              wu[0:P, k0 * W2:k1 * W2])
                          ci += 1
                      eng = nc.sync if k % 2 == 0 else nc.scalar
                      if TT > 1:
                          eng.dma_start(xt[:, k * C:k * C + nb],
                                        xT[:, k * C:k * C + nb])
                      else:
                          eng.dma_start(xt[:, k * C:(k + 1) * C],
                                        xT[:, k * C:(k + 1) * C])
                  if TT > 1:
                      # t>=1 remainder of every k-slice, two bulk strided DMAs
                      KH = KT // 2
                      nc.sync.dma_start(
                          k3(xt[:, :])[:, 0:KH, nb:C],
                          k3(xT[:, :])[:, 0:KH, nb:C])
                      nc.scalar.dma_start(
                          k3(xt[:, :])[:, KH:KT, nb:C],
                          k3(xT[:, :])[:, KH:KT, nb:C])

                  for p in range(IP):
                    if p == 0:
                        wgp, wup = wgp0, wup0
                        # pair-0 groups run t-outer: all t=0 work first,
                        # matching the DMA arrival order above
                        jts = [(j, t) for t in range(TT) for j in range(2)]
                    else:
                        wgp = w1.tile([P, KT * W2], bf16, tag="wg")
                        nc.sync.dma_start(wgp[:, :], wg[p * P:(p + 1) * P, :])
                        wup = w1.tile([P, KT * W2], bf16, tag="wu")
                        nc.scalar.dma_start(wup[:, :], wu[p * P:(p + 1) * P, :])
                        jts = [(j, t) for j in range(2) for t in range(TT)]
                        if p == 4:
                            # prefetch Wd h=0 once the startup ramp has
                            # drained; phase 2 needs it at ~2/3 of the span
                            for d in range(2):
                                kk = IT // 2
                                eng = nc.sync if d % 2 == 0 else nc.scalar
                                eng.dma_start(
                                    wd0[:, d * kk * nb:(d + 1) * kk * nb],
                                    wd[0:P, d * kk * nb:(d + 1) * kk * nb])
                    for j, t in jts:
                        i = 2 * p + j
                        psg = ps1.tile([P, nb], f32, tag=f"g{t}")
                        psu = ps1.tile([P, nb], f32, tag=f"u{t}", bufs=1)
                        for k in range(KT):
                            rhs = xt[:, k * C + t * nb: k * C + t * nb + nb]
                            lhs = wgp[:, k * W2 + j * P: k * W2 + (j + 1) * P]
                            nc.tensor.matmul(psg[:], lhs, rhs,
                                             start=(k == 0), stop=(k == KT - 1))
                        for k in range(KT):
                            rhs = xt[:, k * C + t * nb: k * C + t * nb + nb]
                            lhs = wup[:, k * W2 + j * P: k * W2 + (j + 1) * P]
                            nc.tensor.matmul(psu[:], lhs, rhs,
                                             start=(k == 0), stop=(k == KT - 1))
                        # silu(g)*u = sigmoid(g)*g*u; each DVE op may
                        # read at most ONE operand from PSUM.
                        sig = tmp1.tile([P, nb], f32, tag="sig")
                        nc.scalar.activation(
                            sig[:], psg[:], mybir.ActivationFunctionType.Sigmoid)
                        sg = tmp1.tile([P, nb], f32, tag="sg")
                        nc.vector.tensor_mul(sg[:], sig[:], psg[:])
                        nc.vector.tensor_mul(
                            interT[:, i * C + t * nb: i * C + t * nb + nb],
                            sg[:], psu[:])

              # ---------------- Phase 2: down GEMM ----------------
              with tc.tile_pool(name="w2", bufs=2) as w2, \
                 tc.tile_pool(name="ps2", bufs=2, space="PSUM") as ps2, \
                 tc.tile_pool(name="ot2", bufs=4) as ot2:
                  for h in range(HB):
                    if h == 0:
                        wdh = wd0
                    else:
                        wdh = w2.tile([P, IT * nb], bf16, tag="wd")
                        for d in range(2):
                            kk = IT // 2
                            eng = nc.sync if d % 2 == 0 else nc.scalar
                            eng.dma_start(
                                wdh[:, d * kk * nb:(d + 1) * kk * nb],
                                wd[h * P:(h + 1) * P,
                                   d * kk * nb:(d + 1) * kk * nb])
                    for t in range(T8):
                        ps = ps2.tile([P, nb], f32, tag="o")
                        for k in range(IT):
                            nc.tensor.matmul(
                                ps[:],
                                interT[:, k * C + t * P: k * C + t * P + P],
                                wdh[:, k * nb:(k + 1) * nb],
                                start=(k == 0), stop=(k == IT - 1))
                        ot = ot2.tile([P, nb], bf16, tag="ot")
                        nc.scalar.copy(ot[:], ps[:])
                        nc.sync.dma_start(out[t * P:(t + 1) * P, h * nb:(h + 1) * nb], ot[:])

    nc.compile()
    return nc


def _get_nc(C, Hd, Id):
    key = (C, Hd, Id)
    if key not in _NC_CACHE:
        _NC_CACHE[key] = _build(C, Hd, Id)
    return _NC_CACHE[key]


def _tile_x(xe, Hd, C):
    """[C, Hd] fp32 -> [P, KT*C] bf16 with xT[p, k*C+c] = x[c, k*P+p]."""
    KT = Hd // P
    t = xe.T.reshape(KT, P, C).transpose(1, 0, 2).reshape(P, KT * C)
    return np.ascontiguousarray(t).astype(ml_dtypes.bfloat16)


def _tile_w1(w, Hd, Id):
    """[Hd, Id] fp32 -> [IP*P, KT*W2] bf16 (pair-tiled gate/up layout)."""
    KT = Hd // P
    IP = Id // (2 * P)
    W2 = 2 * P
    t = w.reshape(KT, P, IP, W2).transpose(2, 1, 0, 3).reshape(IP * P, KT * W2)
    return np.ascontiguousarray(t).astype(ml_dtypes.bfloat16)


def _tile_wd(w, Id, Hd, nb=NB):
    """[Id, Hd] fp32 -> [HB*P, IT*nb] bf16 (h-block-tiled down layout)."""
    IT = Id // P
    HB = Hd // nb
    t = w.reshape(IT, P, HB, nb).transpose(2, 1, 0, 3).reshape(HB * P, IT * nb)
    return np.ascontiguousarray(t).astype(ml_dtypes.bfloat16)


def _prepare(inputs):
    """Host-side dispatch: split tokens by expert, pad to capacity, cast to
    bf16, and pre-tile everything into the device SBUF layouts so all DMAs
    are contiguous."""
    x = np.asarray(inputs["permuted_local_hidden_states"], dtype=np.float32)
    tpe = np.asarray(inputs["tokens_per_expert"], dtype=np.int64)
    gate = np.asarray(inputs["gate_proj"], dtype=np.float32)
    up = np.asarray(inputs["up_proj"], dtype=np.float32)
    down = np.asarray(inputs["down_proj"], dtype=np.float32)

    Ee, Hd, Id = gate.shape
    Tt = x.shape[0]
    assert Ee == E, f"expected {E} experts, got {Ee}"
    counts = [int(c) for c in tpe]
    starts = [0]
    for c in counts:
        starts.append(starts[-1] + c)
    cmax = max(max(counts), 1)
    # round capacity to a multiple of NB so TT = C//NB tiles exactly
    C = max(C_DEFAULT, ((cmax + NB - 1) // NB) * NB)

    in_maps = []
    for e in range(Ee):
        s, cnt = starts[e], counts[e]
        if cnt == C:
            xe = x[s:s + cnt]
        else:
            xe = np.zeros((C, Hd), np.float32)
            xe[:cnt] = x[s:s + cnt]
        in_maps.append({
            "xT": _tile_x(xe, Hd, C),
            "wg": _tile_w1(gate[e], Hd, Id),
            "wu": _tile_w1(up[e], Hd, Id),
            "wd": _tile_wd(down[e], Id, Hd),
        })
    meta = (Tt, Hd, starts, counts, C)
    return in_maps, meta


def _postprocess(results, meta):
    Tt, Hd, starts, counts, _C = meta
    outf = np.zeros((Tt, Hd), np.float32)
    for e in range(len(counts)):
        s, cnt = starts[e], counts[e]
        if cnt > 0:
            outf[s:s + cnt] = np.asarray(results[e]["out"])[:cnt].astype(np.float32)
    return outf


def kernel(**inputs):
    from concourse.bass_utils import run_bass_kernel_spmd
    in_maps, meta = _prepare(inputs)
    nc = _get_nc(meta[4], meta[1], np.asarray(inputs["gate_proj"]).shape[2])
    res = run_bass_kernel_spmd(nc, in_maps, list(range(E)))
    return _postprocess(res.results, meta)


# revision 4
# speedup vs baseline: 1.0063x; 1.0063x over previous
"""Grouped MoE MLP (SwiGLU) for Trainium2, expert-parallel across 8 NeuronCores.

Problem: out = gmm(silu(gmm(x,Wg)) * gmm(x,Wu), Wd) with E=8 experts,
T=8192 tokens pre-sorted by expert, H=2048, I=4096.

Strategy: expert parallelism — core e computes expert e's tokens end-to-end.
The host splits the (ragged) token dim by expert, pads each group to a fixed
capacity C, casts to bf16, and PRE-TILES every tensor into the exact SBUF
layout the device consumes, so every DMA is a plain contiguous 2D copy at
full line rate (2-8 KB lines, minimal descriptors, minimal issue cost).

Device program per core (all shapes hardcoded at build time):
  GEMM1 computes the SwiGLU intermediate TRANSPOSED (interT[I, C]) so that
  GEMM2's contraction dim (I) is already the partition dim — no on-device
  transposes anywhere. bf16 inputs, fp32 PSUM accumulation, bf16 output.

Ramp engineering (the steady-state MM stream is already at the 216 ns/MM
N=512 floor, so the wins are at the edges):
  * ~14 warmup matmuls on a zeroed scratch tile run during the DMA ramp so
    the PE_HAM clock gate reaches K=8/8 (2.4 GHz) before real work arrives.
  * The first weight chunk is one k-slice (64 KB) and the first xt slice is
    one (k, t)-quarter (128 KB), so the first real matmul fires ~2 us
    earlier than with coarse transfers.
  * All t=0 halves of xt stream before t=1 halves, alternating rings by k
    parity, and pair-0's psum groups run t-outer — the PE consumes data in
    exactly the order it lands.
"""

import numpy as np
import ml_dtypes

P = 128          # partition dim
NB = 512         # matmul moving free dim / PSUM bank width (fp32)
E, T, H, I = 8, 8192, 2048, 4096
C_DEFAULT = T // E  # per-expert token capacity
WARMUP = 24      # HAM warmup matmuls

_NC_CACHE = {}


def _build(C, Hd, Id, nb=NB):
    """Build + bacc-compile the per-core Tile program. Returns the Bass module."""
    import concourse.bass as bass  # noqa: F401
    import concourse.tile as tile
    from concourse import bacc, mybir

    bf16 = mybir.dt.bfloat16
    f32 = mybir.dt.float32
    KT = Hd // P       # GEMM1 contraction tiles (over H)
    IT = Id // P       # i-tiles (GEMM1 output partitions / GEMM2 contraction)
    TT = C // nb       # token blocks for GEMM1 moving operand
    T8 = C // P        # token tiles for GEMM2 output partitions
    HB = Hd // nb      # h-blocks for GEMM2 moving operand
    IP = IT // 2       # i-block pairs
    W2 = 2 * P         # pair width in I columns

    nc = bacc.Bacc(
        "TRN2",
        target_bir_lowering=False,
        debug=False,
        enable_asserts=False,
        num_devices=8,
    )
    # Host-pre-tiled layouts (see _prepare):
    #   xT[p, k*C + c]   = x[c, k*P + p]
    #   wg[pp*P+p, k*W2+c] = gate[k*P+p, pp*W2+c]   (same for wu)
    #   wd[h*P+p, k*nb+c]  = down[k*P+p, h*nb+c]
    xT = nc.dram_tensor("xT", [P, KT * C], bf16, kind="ExternalInput").ap()
    wg = nc.dram_tensor("wg", [IP * P, KT * W2], bf16, kind="ExternalInput").ap()
    wu = nc.dram_tensor("wu", [IP * P, KT * W2], bf16, kind="ExternalInput").ap()
    wd = nc.dram_tensor("wd", [HB * P, IT * nb], bf16, kind="ExternalInput").ap()
    out = nc.dram_tensor("out", [C, Hd], bf16, kind="ExternalOutput").ap()

    def k3(ap):
        return ap.rearrange("p (k c) -> p k c", c=C)

    with tile.TileContext(nc) as tc:
        with tc.tile_pool(name="res", bufs=1) as res:
            # SwiGLU intermediate, transposed: interT[p, i*C + c] = inter[c, i*P+p]
            interT = res.tile([P, IT * C], bf16)
            # h=0 block of Wd, prefetched during phase 1 so phase 2 starts hot
            wd0 = res.tile([P, IT * nb], bf16)

            # ps1 spans BOTH phases (6 banks) so the allocator must give ps2
            # disjoint fresh banks — otherwise the first phase-2 matmul
            # inherits a conservative wait on ALL phase-1 matmuls completing.
            with tc.tile_pool(name="ps1", bufs=2, space="PSUM") as ps1:
              # ------------- Phase 1: gate/up GEMMs + SwiGLU -------------
              with tc.tile_pool(name="wrm", bufs=1, space="PSUM") as wrm, \
                 tc.tile_pool(name="p1x", bufs=1) as p1x, \
                 tc.tile_pool(name="w1", bufs=3) as w1, \
                 tc.tile_pool(name="tmp1", bufs=4) as tmp1:
                  # HAM warmup: keep the PE busy during the DMA ramp so the
                  # clock gate opens to 2.4 GHz before the first real matmul.
                  scr = p1x.tile([P, P], bf16)
                  nc.vector.memset(scr[:], 0.0)
                  psw = wrm.tile([P, P], f32)
                  for _ in range(WARMUP):
                      nc.tensor.matmul(psw[:], scr[:], scr[:],
                                       start=True, stop=True)

                  wgp0 = w1.tile([P, KT * W2], bf16, tag="wg")
                  wup0 = w1.tile([P, KT * W2], bf16, tag="wu")
                  xt = p1x.tile([P, KT * C], bf16)
                  # Ramp: the first group consumes (wg k-slice, xt k-t0
                  # slice) pairs in k order; feed both rings so every slice
                  # lands just before the PE needs it.  Weight chunks grow
                  # geometrically (k0 | k1 | k2-3 | k4-7 | k8-15).
                  CH = [(0, 1), (1, 2), (2, 4), (4, 8), (8, KT)]
                  ci = 0
                  for k in range(KT):
                      if ci < len(CH) and CH[ci][0] == k:
                          k0, k1 = CH[ci]
                          nc.sync.dma_start(
                              wgp0[:, k0 * W2:k1 * W2],
                              wg[0:P, k0 * W2:k1 * W2])
                          nc.scalar.dma_start(
                              wup0[:, k0 * W2:k1 * W2],
                              wu[0:P, k0 * W2:k1 * W2])
                          ci += 1
                      eng = nc.sync if k % 2 == 0 else nc.scalar
                      if TT > 1:
                          eng.dma_start(xt[:, k * C:k * C + nb],
                                        xT[:, k * C:k * C + nb])
                      else:
                          eng.dma_start(xt[:, k * C:(k + 1) * C],
                                        xT[:, k * C:(k + 1) * C])
                  if TT > 1:
                      # t>=1 remainder of every k-slice rides the otherwise
                      # idle GpSimd DMA ring (4 bulk strided DMAs), keeping
                      # both HW rings free for the weight stream
                      KQ = KT // 4
                      for d in range(4):
                          nc.gpsimd.dma_start(
                              k3(xt[:, :])[:, d * KQ:(d + 1) * KQ, nb:C],
                              k3(xT[:, :])[:, d * KQ:(d + 1) * KQ, nb:C])

                  # pairs 1-2 hoisted ahead of the p-loop (chunked so the
                  # first matmul of each pair gates on half the transfer):
                  # the PE reaches pair 1 at ~28 us and the rings must have
                  # it done by then.
                  hoisted = {}
                  KH = KT // 2
                  for p in (1, 2):
                      if p >= IP:
                          break
                      wgp = w1.tile([P, KT * W2], bf16, tag="wg")
                      wup = w1.tile([P, KT * W2], bf16, tag="wu")
                      for k0, k1 in ((0, KH), (KH, KT)):
                          nc.sync.dma_start(wgp[:, k0 * W2:k1 * W2],
                                            wg[p * P:(p + 1) * P, k0 * W2:k1 * W2])
                          nc.scalar.dma_start(wup[:, k0 * W2:k1 * W2],
                                              wu[p * P:(p + 1) * P, k0 * W2:k1 * W2])
                      hoisted[p] = (wgp, wup)

                  for p in range(IP):
                    if p == 0:
                        wgp, wup = wgp0, wup0
                        # pair-0 groups run t-outer: all t=0 work first,
                        # matching the DMA arrival order above
                        jts = [(j, t) for t in range(TT) for j in range(2)]
                    elif p in hoisted:
                        wgp, wup = hoisted[p]
                        jts = [(j, t) for j in range(2) for t in range(TT)]
                    else:
                        wgp = w1.tile([P, KT * W2], bf16, tag="wg")
                        wup = w1.tile([P, KT * W2], bf16, tag="wu")
                        for k0, k1 in ((0, KH), (KH, KT)):
                            nc.sync.dma_start(wgp[:, k0 * W2:k1 * W2],
                                              wg[p * P:(p + 1) * P, k0 * W2:k1 * W2])
                            nc.scalar.dma_start(wup[:, k0 * W2:k1 * W2],
                                                wu[p * P:(p + 1) * P, k0 * W2:k1 * W2])
                        jts = [(j, t) for j in range(2) for t in range(TT)]
                        if p == 4:
                            # prefetch Wd h=0 once the startup ramp has
                            # drained; phase 2 needs it at ~2/3 of the span
                            for d in range(2):
                                kk = IT // 2
                                eng = nc.sync if d % 2 == 0 else nc.scalar
                                eng.dma_start(
                                    wd0[:, d * kk * nb:(d + 1) * kk * nb],
                                    wd[0:P, d * kk * nb:(d + 1) * kk * nb])
                    for j, t in jts:
                        i = 2 * p + j
                        psg = ps1.tile([P, nb], f32, tag=f"g{t}")
                        psu = ps1.tile([P, nb], f32, tag=f"u{t}", bufs=1)
                        for k in range(KT):
                            rhs = xt[:, k * C + t * nb: k * C + t * nb + nb]
                            lhs = wgp[:, k * W2 + j * P: k * W2 + (j + 1) * P]
                            nc.tensor.matmul(psg[:], lhs, rhs,
                                             start=(k == 0), stop=(k == KT - 1))
                        for k in range(KT):
                            rhs = xt[:, k * C + t * nb: k * C + t * nb + nb]
                            lhs = wup[:, k * W2 + j * P: k * W2 + (j + 1) * P]
                            nc.tensor.matmul(psu[:], lhs, rhs,
                                             start=(k == 0), stop=(k == KT - 1))
                        # silu(g)*u = sigmoid(g)*g*u; each DVE op may
                        # read at most ONE operand from PSUM.
                        sig = tmp1.tile([P, nb], f32, tag="sig")
                        nc.scalar.activation(
                            sig[:], psg[:], mybir.ActivationFunctionType.Sigmoid)
                        sg = tmp1.tile([P, nb], f32, tag="sg")
                        nc.vector.tensor_mul(sg[:], sig[:], psg[:])
                        nc.vector.tensor_mul(
                            interT[:, i * C + t * nb: i * C + t * nb + nb],
                            sg[:], psu[:])

              # ---------------- Phase 2: down GEMM ----------------
              with tc.tile_pool(name="w2", bufs=2) as w2, \
                 tc.tile_pool(name="ps2", bufs=2, space="PSUM") as ps2, \
                 tc.tile_pool(name="ot2", bufs=4) as ot2:
                  for h in range(HB):
                    if h == 0:
                        wdh = wd0
                    else:
                        wdh = w2.tile([P, IT * nb], bf16, tag="wd")
                        for d in range(2):
                            kk = IT // 2
                            eng = nc.sync if d % 2 == 0 else nc.scalar
                            eng.dma_start(
                                wdh[:, d * kk * nb:(d + 1) * kk * nb],
                                wd[h * P:(h + 1) * P,
                                   d * kk * nb:(d + 1) * kk * nb])
                    for t in range(T8):
                        ps = ps2.tile([P, nb], f32, tag="o")
                        for k in range(IT):
                            nc.tensor.matmul(
                                ps[:],
                                interT[:, k * C + t * P: k * C + t * P + P],
                                wdh[:, k * nb:(k + 1) * nb],
                                start=(k == 0), stop=(k == IT - 1))
                        ot = ot2.tile([P, nb], bf16, tag="ot")
                        nc.scalar.copy(ot[:], ps[:])
                        nc.sync.dma_start(out[t * P:(t + 1) * P, h * nb:(h + 1) * nb], ot[:])

    nc.compile()
    return nc


def _get_nc(C, Hd, Id):
    key = (C, Hd, Id)
    if key not in _NC_CACHE:
        _NC_CACHE[key] = _build(C, Hd, Id)
    return _NC_CACHE[key]


def _tile_x(xe, Hd, C):
    """[C, Hd] fp32 -> [P, KT*C] bf16 with xT[p, k*C+c] = x[c, k*P+p]."""
    KT = Hd // P
    t = xe.T.reshape(KT, P, C).transpose(1, 0, 2).reshape(P, KT * C)
    return np.ascontiguousarray(t).astype(ml_dtypes.bfloat16)


def _tile_w1(w, Hd, Id):
    """[Hd, Id] fp32 -> [IP*P, KT*W2] bf16 (pair-tiled gate/up layout)."""
    KT = Hd // P
    IP = Id // (2 * P)
    W2 = 2 * P
    t = w.reshape(KT, P, IP, W2).transpose(2, 1, 0, 3).reshape(IP * P, KT * W2)
    return np.ascontiguousarray(t).astype(ml_dtypes.bfloat16)


def _tile_wd(w, Id, Hd, nb=NB):
    """[Id, Hd] fp32 -> [HB*P, IT*nb] bf16 (h-block-tiled down layout)."""
    IT = Id // P
    HB = Hd // nb
    t = w.reshape(IT, P, HB, nb).transpose(2, 1, 0, 3).reshape(HB * P, IT * nb)
    return np.ascontiguousarray(t).astype(ml_dtypes.bfloat16)


def _prepare(inputs):
    """Host-side dispatch: split tokens by expert, pad to capacity, cast to
    bf16, and pre-tile everything into the device SBUF layouts so all DMAs
    are contiguous."""
    x = np.asarray(inputs["permuted_local_hidden_states"], dtype=np.float32)
    tpe = np.asarray(inputs["tokens_per_expert"], dtype=np.int64)
    gate = np.asarray(inputs["gate_proj"], dtype=np.float32)
    up = np.asarray(inputs["up_proj"], dtype=np.float32)
    down = np.asarray(inputs["down_proj"], dtype=np.float32)

    Ee, Hd, Id = gate.shape
    Tt = x.shape[0]
    assert Ee == E, f"expected {E} experts, got {Ee}"
    counts = [int(c) for c in tpe]
    starts = [0]
    for c in counts:
        starts.append(starts[-1] + c)
    cmax = max(max(counts), 1)
    # round capacity to a multiple of NB so TT = C//NB tiles exactly
    C = max(C_DEFAULT, ((cmax + NB - 1) // NB) * NB)

    in_maps = []
    for e in range(Ee):
        s, cnt = starts[e], counts[e]
        if cnt == C:
            xe = x[s:s + cnt]
        else:
            xe = np.zeros((C, Hd), np.float32)
            xe[:cnt] = x[s:s + cnt]
        in_maps.append({
            "xT": _tile_x(xe, Hd, C),
            "wg": _tile_w1(gate[e], Hd, Id),
            "wu": _tile_w1(up[e], Hd, Id),
            "wd": _tile_wd(down[e], Id, Hd),
        })
    meta = (Tt, Hd, starts, counts, C)
    return in_maps, meta


def _postprocess(results, meta):
    Tt, Hd, starts, counts, _C = meta
    outf = np.zeros((Tt, Hd), np.float32)
    for e in range(len(counts)):
        s, cnt = starts[e], counts[e]
        if cnt > 0:
            outf[s:s + cnt] = np.asarray(results[e]["out"])[:cnt].astype(np.float32)
    return outf


def kernel(**inputs):
    from concourse.bass_utils import run_bass_kernel_spmd
    in_maps, meta = _prepare(inputs)
    nc = _get_nc(meta[4], meta[1], np.asarray(inputs["gate_proj"]).shape[2])
    res = run_bass_kernel_spmd(nc, in_maps, list(range(E)))
    return _postprocess(res.results, meta)
